# revision 21
# baseline (speedup 1.0000x reference)
"""Sparse (log-mask) attention with entmax15 — Trainium2 Bass kernel, v2.

Sharding: 8 cores, core c handles head h=c for both batch rows.  Each core
computes its head's partial c_proj output; host sums the 8 partials and adds
b_proj.

Key design points vs v1:
  - All big matmuls use float32r (1 cyc/row at moving>=256) or fp16 (1 cyc/row).
  - nmask (-30000 on masked) is preloaded into PSUM with an identity matmul;
    score matmuls accumulate on top (start=False) -> no DVE mask-add pass.
  - S is drained PSUM->SBUF as fp16 by ACT; all entmax passes then run in DVE
    2x/4x perf modes.
  - entmax tau solved by 3 frozen-support quadratic iterations (exact on a
    stable support): per iteration DVE computes y=relu(S-tau) plus row stats
    {sum y, sum y^2, count y>0} via accumulate variants, then the quadratic
    sum_supp (S-t)^2 = 4 is solved exactly.  Final att = relu(S-tau)^2 / 4
    (the row sum equals 4 by construction at the solved tau).
  - att is transposed by the DMA xbar (fp16) instead of PE+copy; v likewise.
  - AV and c_proj run in fp16 with av kept [dv, q] so c_proj needs no
    transpose.
"""

import numpy as np
import ml_dtypes

B = 2
S = 2048
D = 128
H = 8
QL = 5
PAD = QL - 1
NEG = -30000.0
NTILE = S // 128  # 16 row tiles
NITER = 3  # frozen-support quadratic iterations

# solver: secant on g(tau) = sqrt(sum relu(S-tau)^2) - 2, E tau-points
# (g evaluated at the first E-1; the final point only feeds the att apply)
NEVAL = 5  # g evaluations: ta, tb, then NEVAL-2 secant updates with re-eval
# sum(y^2) is split by column range: ACT (Square+accum, 1x) takes the first
# ACT_FRAC of each chain's width, DVE (tensor_tensor square + copy-accum,
# 2x+4x) the rest — the halves run concurrently on both engines.
ACT_FRAC = 0.45
TSPLIT = 0  # chains with T < TSPLIT use the ACT Relu for the relu pass

_CACHE = {}

# tile groups: (tiles, batch) — single-batch groups let batch b's solver
# start as soon as batch b's projections exist
GROUPS = [([15, 0, 14, 1], 0), ([13, 2, 12, 3], 0),
          ([11, 4, 10, 5], 0), ([9, 6, 8, 7], 0),
          ([15, 0, 14, 1], 1), ([13, 2, 12, 3], 1),
          ([11, 4, 10, 5], 1), ([9, 6, 8, 7], 1)]


def _build_program(repeat=1):
    import concourse.bass as bass
    import concourse.mybir as mybir
    import concourse.tile as tile
    from concourse import bacc

    f32 = mybir.dt.float32
    f16 = mybir.dt.float16
    bf16 = mybir.dt.bfloat16

    nc = bacc.Bacc("TRN2", target_bir_lowering=False, debug=False,
                   enable_asserts=False)

    f32r_ = mybir.dt.float32r
    f16_ = mybir.dt.float16
    x_d = nc.dram_tensor("x", [B, S, D], f16_, kind="ExternalInput").ap()
    wq_d = nc.dram_tensor("wq", [QL, D, D], f16_, kind="ExternalInput").ap()
    wk_d = nc.dram_tensor("wk", [QL, D, D], f16_, kind="ExternalInput").ap()
    bq_d = nc.dram_tensor("bq", [D, 1], f32, kind="ExternalInput").ap()
    bk_d = nc.dram_tensor("bk", [D, 1], f32, kind="ExternalInput").ap()
    wv_d = nc.dram_tensor("wv", [D, D], f16_, kind="ExternalInput").ap()
    bv_d = nc.dram_tensor("bv", [D, 1], f32, kind="ExternalInput").ap()
    wp_d = nc.dram_tensor("wp", [D, D], f16, kind="ExternalInput").ap()
    nm_d = nc.dram_tensor("nmask", [S, S], bf16, kind="ExternalInput").ap()
    po_d = nc.dram_tensor("po", [B, D, S], f32, kind="ExternalOutput").ap()

    with tile.TileContext(nc) as tc:
        for _rep in range(repeat):
            _body(nc, tc, tile, mybir,
                  x_d, wq_d, wk_d, bq_d, bk_d, wv_d, bv_d, wp_d, nm_d, po_d)
    nc.compile()
    return nc


def _body(nc, tc, tile, mybir,
          x_d, wq_d, wk_d, bq_d, bk_d, wv_d, bv_d, wp_d, nm_d, po_d):
    from contextlib import ExitStack
    from concourse.bass import ts
    from concourse.masks import make_identity

    f32 = mybir.dt.float32
    f32r = mybir.dt.float32r
    f16 = mybir.dt.float16
    bf16 = mybir.dt.bfloat16
    AF = mybir.ActivationFunctionType
    OP = mybir.AluOpType
    AX = mybir.AxisListType.X

    ctx = ExitStack()
    with ctx:
        cpool = ctx.enter_context(tc.tile_pool(name="consts", bufs=1))
        xpool = ctx.enter_context(tc.tile_pool(name="xn", bufs=2))
        qkvp = ctx.enter_context(tc.tile_pool(name="qkv", bufs=2))
        vtp = ctx.enter_context(tc.tile_pool(name="vt", bufs=1))
        # wide (T>=8, W up to 2048) and narrow (T<8, W<=1024) pool variants
        spool_w = ctx.enter_context(tc.tile_pool(name="scw", bufs=8))
        spool_n = ctx.enter_context(tc.tile_pool(name="scn", bufs=8))
        ypool_w = ctx.enter_context(tc.tile_pool(name="ybw", bufs=5))
        ypool_n = ctx.enter_context(tc.tile_pool(name="ybn", bufs=5))
        attp_w = ctx.enter_context(tc.tile_pool(name="attw", bufs=2))
        attp_n = ctx.enter_context(tc.tile_pool(name="attn", bufs=2))
        atp_w = ctx.enter_context(tc.tile_pool(name="attTw", bufs=2))
        atp_n = ctx.enter_context(tc.tile_pool(name="attTn", bufs=1))
        avp = ctx.enter_context(tc.tile_pool(name="avs", bufs=1))
        pop = ctx.enter_context(tc.tile_pool(name="pot", bufs=2))
        nmp_w = ctx.enter_context(tc.tile_pool(name="nmw", bufs=3))
        nmp_n = ctx.enter_context(tc.tile_pool(name="nmn", bufs=3))
        stp = ctx.enter_context(tc.tile_pool(name="st", bufs=30))

        def wtile(pool_w, pool_n, T, dt_, tag):
            if T >= 8:
                t_ = pool_w.tile([128, S], dt_, tag=tag + "w")
            else:
                t_ = pool_n.tile([128, S // 2], dt_, tag=tag + "n")
            return t_
        ps_sc = ctx.enter_context(tc.tile_pool(name="pssc", bufs=2, space="PSUM"))
        ps_cv = ctx.enter_context(tc.tile_pool(name="pscv", bufs=1, space="PSUM"))
        ps_av = ctx.enter_context(tc.tile_pool(name="psav", bufs=2, space="PSUM"))
        ps_pj = ctx.enter_context(tc.tile_pool(name="pspj", bufs=1, space="PSUM"))

        ident = cpool.tile([128, 128], f32, tag="ident")
        make_identity(nc, ident)
        identh = cpool.tile([128, 128], bf16, tag="identh")
        nc.vector.tensor_copy(identh[:], ident[:])

        wq_sb = cpool.tile([128, QL * 128], f16, tag="wq")
        wk_sb = cpool.tile([128, QL * 128], f16, tag="wk")
        for t in range(QL):
            nc.sync.dma_start(wq_sb[:, ts(t, 128)], wq_d[t])
            nc.sync.dma_start(wk_sb[:, ts(t, 128)], wk_d[t])
        wv_sb = cpool.tile([128, 128], f16, tag="wv")
        wp_sb = cpool.tile([128, 128], f16, tag="wp")
        nc.sync.dma_start(wv_sb[:], wv_d[:])
        nc.sync.dma_start(wp_sb[:], wp_d[:])
        bq_sb = cpool.tile([128, 1], f32, tag="bq")
        bk_sb = cpool.tile([128, 1], f32, tag="bk")
        bv_sb = cpool.tile([128, 1], f32, tag="bv")
        nc.sync.dma_start(bq_sb[:], bq_d[:])
        nc.sync.dma_start(bk_sb[:], bk_d[:])
        nc.sync.dma_start(bv_sb[:], bv_d[:])

        # per-batch projections: qT/kT fp16 [d,s], v_nat fp16 [s,dv] chunked
        qT, kT, v_nat, poT = [None] * B, [None] * B, [None] * B, [None] * B

        def emit_qkv(b):
            # x fp16, transposed to xT [d, s] by the DMA xbar
            xt = xpool.tile([128, S], f16, tag="xt")
            nc.sync.dma_start_transpose(xt[:], x_d[b])

            qTb = qkvp.tile([128, S], f16, tag="qT")
            kTb = qkvp.tile([128, S], f16, tag="kT")
            vTb = vtp.tile([128, S], f16, tag="vT")
            for n in range(S // 512):
                for (dst, w_sb, b_sb) in ((qTb, wq_sb, bq_sb),
                                          (kTb, wk_sb, bk_sb)):
                    pq = ps_cv.tile([128, 512], f32, tag="pscv")
                    # tap order: shift-0 first (full range, start=True); the
                    # shifted taps clip their output range at the left edge
                    for t in range(QL - 1, -1, -1):
                        sh = QL - 1 - t
                        lo = max(0, sh - n * 512)
                        nc.tensor.matmul(
                            pq[:, lo:512],
                            w_sb[:, ts(t, 128)],
                            xt[:, n * 512 + lo - sh: n * 512 + 512 - sh],
                            start=(t == QL - 1), stop=(t == 0))
                    nc.scalar.activation(dst[:, ts(n, 512)], pq[:],
                                         AF.Identity, bias=b_sb[:])
                pv = ps_cv.tile([128, 512], f32, tag="pscv")
                nc.tensor.matmul(pv[:], wv_sb[:],
                                 xt[:, n * 512:(n + 1) * 512],
                                 start=True, stop=True)
                nc.scalar.activation(vTb[:, ts(n, 512)], pv[:], AF.Identity,
                                     bias=bv_sb[:])
            vn = qkvp.tile([128, S], f16, tag="vnat")
            nc.sync.dma_start_transpose(
                vn[:].rearrange("p (c k) -> p c k", c=NTILE), vTb[:])
            qT[b] = qTb
            kT[b] = kTb
            v_nat[b] = vn
            po_b = pop.tile([128, S], f32, tag="pot")
            poT[b] = po_b

        class Grp:
            pass

        def emit_scores(group):
            tiles, gb_ = group
            st = Grp()
            st.chains = [(T, gb_) for T in tiles]
            st.G = len(st.chains)
            st.Mv = stp.tile([128, st.G], f32, tag="st")
            st.s_list = []
            nm_cache = {}
            for c, (T, b) in enumerate(st.chains):
                W = (T + 1) * 128
                if T not in nm_cache:
                    nm = wtile(nmp_w, nmp_n, T, bf16, "nm")
                    nc.sync.dma_start(nm[:, 0:W], nm_d[ts(T, 128), 0:W])
                    nm_cache[T] = nm
                nm = nm_cache[T]
                s_sb = wtile(spool_w, spool_n, T, f16, "Ssb")
                for kc in range(0, W, 1024):
                    cw = min(1024, W - kc)
                    pqk = ps_sc.tile([128, 1024], f32, tag="ps")
                    for sub in range(0, cw, 512):
                        sw = min(512, cw - sub)
                        nc.tensor.matmul(
                            pqk[:, sub:sub + sw], identh[:],
                            nm[:, kc + sub:kc + sub + sw],
                            start=True, stop=False)
                        nc.tensor.matmul(
                            pqk[:, sub:sub + sw], qT[b][:, ts(T, 128)],
                            kT[b][:, kc + sub:kc + sub + sw],
                            start=False, stop=True)
                    nc.scalar.activation(s_sb[:, kc:kc + cw], pqk[:, 0:cw],
                                         AF.Identity)
                # rowmax via copy-with-accum(max): 4x mode vs 1x tensor_reduce
                mjunk = wtile(ypool_w, ypool_n, T, f16, "yb")
                nc.vector.tensor_scalar(
                    out=mjunk[:, 0:W], in0=s_sb[:, 0:W], scalar1=1.0,
                    scalar2=NEG, op0=OP.mult, op1=OP.max,
                    accum_out=st.Mv[:, c:c + 1])
                st.s_list.append(s_sb)
            G = st.G
            st.ta = stp.tile([128, G], f32, tag="st")
            st.tb = stp.tile([128, G], f32, tag="st")
            st.nta = stp.tile([128, G], f32, tag="st")
            st.ntb = stp.tile([128, G], f32, tag="st")
            nc.vector.tensor_scalar_add(st.ta[:], st.Mv[:], -2.0)
            nc.vector.tensor_scalar_add(st.tb[:], st.Mv[:], -1.0)
            nc.vector.tensor_scalar_mul(st.nta[:], st.ta[:], -1.0)
            nc.vector.tensor_scalar_mul(st.ntb[:], st.tb[:], -1.0)
            return st

        def relu_pass(st, c, tcur, ntcur):
            T, b = st.chains[c]
            W = (T + 1) * 128
            y = wtile(ypool_w, ypool_n, T, f16, "yb")
            if T < TSPLIT:
                nc.scalar.activation(y[:, 0:W], st.s_list[c][:, 0:W], AF.Relu,
                                     bias=ntcur[:, c:c + 1])
            else:
                nc.vector.tensor_scalar(
                    out=y[:, 0:W], in0=st.s_list[c][:, 0:W],
                    scalar1=tcur[:, c:c + 1], scalar2=ntcur[:, c:c + 1],
                    op0=OP.max, op1=OP.add)
            return y

        def csplit(W, T):
            # parity split: odd-T chains run the whole Sy2 pass on ACT,
            # even-T on DVE (53/47 area split, one instruction per chain)
            return W if T % 2 == 1 else 0

        def emit_eval(st, tcur, ntcur):
            syy = stp.tile([128, st.G], f32, tag="st")
            for c, (T, b) in enumerate(st.chains):
                W = (T + 1) * 128
                wa = csplit(W, T)
                if wa > 0:
                    y = relu_pass(st, c, tcur, ntcur)
                    nc.scalar.activation(y[:, 0:wa], y[:, 0:wa], AF.Square,
                                         accum_out=syy[:, c:c + 1])
                else:
                    y = relu_pass(st, c, tcur, ntcur)
                    nc.vector.tensor_tensor(y[:, 0:W], y[:, 0:W],
                                            y[:, 0:W], OP.mult)
                    nc.vector.tensor_scalar(
                        out=y[:, 0:W], in0=y[:, 0:W], scalar1=1.0,
                        scalar2=0.0, op0=OP.mult, op1=OP.add,
                        accum_out=syy[:, c:c + 1])
            g = stp.tile([128, st.G], f32, tag="st")
            nc.scalar.activation(g[:], syy[:], AF.Sqrt)
            nc.vector.tensor_scalar_add(g[:], g[:], -2.0)
            return g

        def emit_secant(st):
            G = st.G
            dt = stp.tile([128, G], f32, tag="st")
            dg = stp.tile([128, G], f32, tag="st")
            tn = stp.tile([128, G], f32, tag="st")
            ntn = stp.tile([128, G], f32, tag="st")
            nc.vector.tensor_tensor(dt[:], st.tb[:], st.ta[:], OP.subtract)
            nc.vector.tensor_tensor(dg[:], st.gb[:], st.ga[:], OP.subtract)
            nc.vector.tensor_scalar_add(dg[:], dg[:], -1e-12)
            nc.vector.reciprocal(dg[:], dg[:])
            nc.vector.tensor_tensor(dt[:], dt[:], dg[:], OP.mult)
            nc.vector.tensor_scalar(out=dt[:], in0=dt[:], scalar1=-1e-9,
                                    scalar2=-4.0, op0=OP.min, op1=OP.max)
            nc.vector.tensor_tensor(tn[:], st.gb[:], dt[:], OP.mult)
            nc.vector.tensor_tensor(tn[:], st.tb[:], tn[:], OP.subtract)
            nc.vector.tensor_scalar_mul(ntn[:], tn[:], -1.0)
            st.ta, st.ga = st.tb, st.gb
            st.tb, st.ntb = tn, ntn

        def emit_final(st):
            # att = relu(S - tau)^2 in fp16 (1/4 folded into wp); AV; c_proj
            for c0 in range(0, st.G, 4):
                H4 = min(4, st.G - c0)
                pav = ps_av.tile([128, 512], f32, tag="av")
                for cc in range(H4):
                    c = c0 + cc
                    T, b = st.chains[c]
                    W = (T + 1) * 128
                    y = relu_pass(st, c, st.tb, st.ntb)
                    att = wtile(attp_w, attp_n, T, f16, "att")
                    if T % 2 == 1:
                        nc.scalar.activation(att[:, 0:W], y[:, 0:W], AF.Square)
                    else:
                        nc.vector.tensor_tensor(att[:, 0:W], y[:, 0:W],
                                                y[:, 0:W], OP.mult)
                    attT = wtile(atp_w, atp_n, T, f16, "attT")
                    nc.sync.dma_start_transpose(
                        attT[:, 0:W].rearrange("p (c k) -> p c k", c=T + 1),
                        att[:, 0:W])
                    for j in range(T + 1):
                        nc.tensor.matmul(pav[:, ts(cc, 128)],
                                         v_nat[b][:, ts(j, 128)],
                                         attT[:, ts(j, 128)],
                                         start=(j == 0), stop=(j == T))
                av_sb = avp.tile([128, 512], f16, tag="avs")
                nc.vector.tensor_copy(av_sb[:], pav[:])
                ppj = ps_pj.tile([128, 512], f32, tag="pj")
                nc.tensor.matmul(ppj[:], wp_sb[:], av_sb[:], start=True,
                                 stop=True)
                for cc in range(H4):
                    c = c0 + cc
                    T, b = st.chains[c]
                    nc.vector.tensor_copy(poT[b][:, ts(T, 128)],
                                          ppj[:, ts(cc, 128)])

        # two groups in lockstep: alternate emission at every eval/secant
        # phase so each engine's in-order queue interleaves independent
        # work — one group's round-trip stalls are filled by the other's
        emit_qkv(0)
        for p0 in range(0, len(GROUPS), 2):
            if p0 == 2:
                emit_qkv(1)
            sts = [emit_scores(GROUPS[p0]), emit_scores(GROUPS[p0 + 1])]
            for st in sts:
                st.ga = emit_eval(st, st.ta, st.nta)
            for st in sts:
                st.gb = emit_eval(st, st.tb, st.ntb)
            for ev in range(2, NEVAL + 1):
                for st in sts:
                    emit_secant(st)
                if ev < NEVAL:
                    for st in sts:
                        st.gb = emit_eval(st, st.tb, st.ntb)
            for st in sts:
                emit_final(st)

        for b in range(B):
            nc.sync.dma_start(po_d[b], poT[b][:])


def _get_program():
    if "nc" not in _CACHE:
        _CACHE["nc"] = _build_program()
    return _CACHE["nc"]


def _make_in_maps(x, mask, w_qk, b_qk, w_v, b_v, w_proj):
    x = np.asarray(x, np.float32)
    mask2d = np.asarray(mask, np.float32).reshape(S, S)
    w_qk = np.asarray(w_qk, np.float32)
    b_qk = np.asarray(b_qk, np.float32)
    w_v = np.asarray(w_v, np.float32)
    b_v = np.asarray(b_v, np.float32)
    w_proj = np.asarray(w_proj, np.float32)
    scale = np.float32(1.0 / np.sqrt(D))
    nmask = ((1.0 - mask2d) * NEG).astype(ml_dtypes.bfloat16)
    in_maps = []
    for c in range(H):
        qs = slice(c * D, (c + 1) * D)
        ks = slice(H * D + c * D, H * D + (c + 1) * D)
        wq = np.ascontiguousarray(
            np.transpose(w_qk[qs], (2, 1, 0))) * scale      # [QL, d_in, d_out]
        wk = np.ascontiguousarray(np.transpose(w_qk[ks], (2, 1, 0)))
        in_maps.append({
            "x": x.astype(np.float16),
            "wq": wq.astype(np.float16),
            "wk": wk.astype(np.float16),
            "bq": (b_qk[qs] * scale).reshape(D, 1).astype(np.float32),
            "bk": b_qk[ks].reshape(D, 1).astype(np.float32),
            "wv": np.ascontiguousarray(w_v[:, qs]).astype(np.float16),
            "bv": b_v[qs].reshape(D, 1).astype(np.float32),
            "wp": (np.ascontiguousarray(w_proj[qs]) * 0.25).astype(np.float16),
            "nmask": nmask,
        })
    return in_maps


def kernel(x, mask, w_qk, b_qk, w_v, b_v, w_proj, b_proj, **_):
    from concourse import bass_utils

    nc = _get_program()
    in_maps = _make_in_maps(x, mask, w_qk, b_qk, w_v, b_v, w_proj)
    res = bass_utils.run_bass_kernel_spmd(nc, in_maps, core_ids=list(range(H)))
    acc = np.zeros((B, D, S), np.float64)
    for r in res.results:
        acc += r["po"].astype(np.float64)
    out = acc.transpose(0, 2, 1).astype(np.float32) + np.asarray(
        b_proj, np.float32)[None, None, :]
    return out
